# revision 21
# baseline (speedup 1.0000x reference)
"""ArcFace-style per-class loss kernel for 8 Trainium2 NeuronCores.

Math (algebraically exact reduction of the reference):
  Xn_i  = X_i / ||X_i||
  sums_c = sum_{i: l_i=c} Xn_i               [C, D] segment sum
  counts_c = |{i: l_i=c}|
  loss_c = (S_c * lse_seg_c - ||sums_c||) / max(counts_c, 1)
    with S_c = colsum_c/||sums_c||, colsum_c = sum_d sums_c[d]
  Because rows are unit-norm, lse_i = log(D + 1/2 + sum_d Xn_id) + O(1e-5)
  (2nd-order Taylor of logsumexp using sum_d Xn^2 = 1), so
  lse_seg_c = K*counts_c + colsum_c/(D+1/2),  K = log(D+1/2).

Sharding: rows are routed (on host) to the core owning their label octant
(core k owns classes [128k, 128k+128)), so every per-class reduction is
fully local to one core — no collectives.  Host also lays X out so each
partition's group data is contiguous in DRAM (16 KB reads).

Per 128-row tile: row sum-of-squares on ACT (Square+accumulate) or DVE
(scalar_tensor_tensor), balanced so both engines stay under the DMA
budget; rsqrt via sqrt+reciprocal+Newton (batched per group); scaled
one-hot = (iota==label)*rnorm in one fused DVE tensor_scalar; bf16 cast of
X is one group-wide DVE copy; PE accumulates sums (one-hotT @ Xbf) and
compensated counts (one-hotT @ (ss*rnorm)) into PSUM across all tiles.
Padded rows have label -1 (zero one-hot column) and X = 0.
"""

import sys

if "/opt/trn_rl_repo" not in sys.path:
    sys.path.insert(0, "/opt/trn_rl_repo")

import math

import numpy as np

import concourse.bass as bass  # noqa: F401
import concourse.tile as tile
from concourse import bacc, mybir
from concourse.bass_utils import run_bass_kernel_spmd

# Problem constants (hardcoded per spec: N=131072, D=512, C=1024, 8 cores)
N_ROWS = 131072
D = 512
C = 1024
NCORES = 8
CLOC = C // NCORES  # 128 classes per core

# Classes are assigned to cores by balanced greedy bin-packing (128 classes
# per core, near-equal row totals), so per-core rows ~ N/8 = 16384 +- ~16.
# Capacity 16640 = 16 full groups of 1024 rows + one 2-tile tail group.
CAP = 16640
P = 128  # partitions / rows per tile
NT = CAP // P  # 130 tiles
G = 8  # tiles per full group (one DMA per group)
NG = 16  # full groups
G_TAIL = 2  # tiles in the tail group

K_CONST = math.log(D + 0.5)
INV_D5 = 1.0 / (D + 0.5)

F32 = mybir.dt.float32
BF16 = mybir.dt.bfloat16


def build_nc():
    nc = bacc.Bacc(None, target_bir_lowering=False)

    x_ext = nc.declare_dram_parameter("x", [NG, P, G, D], F32, isOutput=False)
    xt_ext = nc.declare_dram_parameter("xt", [P, G_TAIL, D], F32, isOutput=False)
    lab_ext = nc.declare_dram_parameter("lab", [P, NT], F32, isOutput=False)
    iota_ext = nc.declare_dram_parameter("iota", [P, CLOC], F32, isOutput=False)
    out_ext = nc.declare_dram_parameter("out", [P, 1], F32, isOutput=True)

    AF = mybir.ActivationFunctionType
    OP = mybir.AluOpType

    with tile.TileContext(nc) as tc:
        with (
            tc.tile_pool(name="xpool", bufs=4) as xpool,
            tc.tile_pool(name="ohpool", bufs=8) as ohpool,
            tc.tile_pool(name="small", bufs=6) as small,
            tc.tile_pool(name="singles", bufs=1) as singles,
            tc.tile_pool(name="psum", bufs=1, space="PSUM") as psum,
        ):
            lab_sb = singles.tile([P, NT], F32)
            nc.sync.dma_start(out=lab_sb[:], in_=lab_ext[:, :])
            iota_sb = singles.tile([P, CLOC], F32)
            nc.sync.dma_start(out=iota_sb[:], in_=iota_ext[:, :])

            # prefetch the sqrt activation table while the first DMAs run
            warm = singles.tile([P, 1], F32)
            nc.vector.memset(warm[:], 1.0)
            nc.scalar.activation(out=warm[:], in_=warm[:], func=AF.Sqrt)

            psum_sums = psum.tile([P, D], F32)  # one full bank
            psum_cnt = psum.tile([P, 1], F32)
            act_scratch = psum.tile([P, D], F32)  # ACT Square dump
            dve_scratch = singles.tile([P, D], F32)  # DVE stt dump

            def process_group(g, t_base, src_ap, gg, n_dve):
                xg = xpool.tile([P, gg, D], F32, tag="xg", name=f"xg{g}")
                nc.sync.dma_start(out=xg[:], in_=src_ap)

                xbf = xpool.tile([P, gg, D], BF16, tag="xbf", name=f"xbf{g}")
                nc.gpsimd.tensor_copy(xbf[:], xg[:])

                # per-row sum of squares, split ACT / DVE to balance load
                ssg = small.tile([P, gg], F32, tag="ssg", name=f"ssg{g}")
                for j in range(gg):
                    if j >= gg - n_dve:
                        nc.vector.scalar_tensor_tensor(
                            out=dve_scratch[:],
                            in0=xg[:, j],
                            scalar=1.0,
                            in1=xg[:, j],
                            op0=OP.mult,
                            op1=OP.mult,
                            accum_out=ssg[:, j : j + 1],
                        )
                    else:
                        nc.scalar.activation(
                            out=act_scratch[:],
                            in_=xg[:, j],
                            func=AF.Square,
                            accum_out=ssg[:, j : j + 1],
                        )

                # rnorm = 1/sqrt(max(ss, eps)), Newton-refined; ncol = ss*rnorm
                def st(nm):
                    return small.tile([P, gg], F32, tag=nm, name=f"{nm}{g}")

                ssc = st("ssc")
                nc.vector.tensor_scalar_max(ssc[:], ssg[:], 1e-12)
                sqg = st("sqg")
                nc.scalar.activation(out=sqg[:], in_=ssc[:], func=AF.Sqrt)
                r0 = st("r0")
                nc.vector.reciprocal(r0[:], sqg[:])
                t0 = st("t0")
                nc.vector.tensor_mul(t0[:], r0[:], r0[:])
                t1 = st("t1")
                nc.vector.tensor_mul(t1[:], t0[:], ssc[:])
                t2 = st("t2")
                nc.vector.tensor_scalar(t2[:], t1[:], -0.5, 1.5, OP.mult, OP.add)
                rn = st("rn")
                nc.vector.tensor_mul(rn[:], r0[:], t2[:])
                ncol = st("ncol")
                nc.vector.tensor_mul(ncol[:], ssc[:], rn[:])
                ncbf = small.tile([P, gg], BF16, tag="ncbf", name=f"ncbf{g}")
                nc.vector.tensor_copy(ncbf[:], ncol[:])

                for j in range(gg):
                    t = t_base + j
                    oh = ohpool.tile([P, CLOC], BF16, tag="oh", name=f"oh{t}")
                    nc.vector.tensor_scalar(
                        oh[:],
                        iota_sb[:],
                        lab_sb[:, t : t + 1],
                        rn[:, j : j + 1],
                        OP.is_equal,
                        OP.mult,
                    )
                    nc.tensor.matmul(
                        psum_sums[:],
                        lhsT=oh[:],
                        rhs=xbf[:, j],
                        start=(t == 0),
                        stop=(t == NT - 1),
                    )
                    nc.tensor.matmul(
                        psum_cnt[:],
                        lhsT=oh[:],
                        rhs=ncbf[:, j : j + 1],
                        start=(t == 0),
                        stop=(t == NT - 1),
                    )

            for g in range(NG):
                process_group(
                    g, g * G, x_ext[g], G, n_dve=(1 if (g % 2 == 0) else 2)
                )
            process_group(NG, NG * G, xt_ext[:, :, :], G_TAIL, n_dve=1)

            # ---- epilogue: per-class loss from sums/counts ----
            sums_sb = singles.tile([P, D], F32)
            nc.vector.tensor_copy(sums_sb[:], psum_sums[:])
            cnt = singles.tile([P, 1], F32)
            nc.vector.tensor_copy(cnt[:], psum_cnt[:])

            colsum = singles.tile([P, 1], F32)
            nc.vector.tensor_reduce(
                colsum[:], sums_sb[:], mybir.AxisListType.X, OP.add
            )
            junk = singles.tile([P, D], F32)
            nc.vector.tensor_mul(junk[:], sums_sb[:], sums_sb[:])
            sumsq = singles.tile([P, 1], F32)
            nc.vector.tensor_reduce(
                sumsq[:], junk[:], mybir.AxisListType.X, OP.add
            )

            _ep_n = [0]

            def newt():
                _ep_n[0] += 1
                return singles.tile(
                    [P, 1], F32, name=f"ep{_ep_n[0]}", tag=f"ep{_ep_n[0]}"
                )

            s0 = newt()
            nc.vector.tensor_scalar_max(s0[:], sumsq[:], 1e-20)
            sq2 = newt()
            nc.scalar.activation(out=sq2[:], in_=s0[:], func=AF.Sqrt)
            r0e = newt()
            nc.vector.reciprocal(r0e[:], sq2[:])
            a0 = newt()
            nc.vector.tensor_mul(a0[:], r0e[:], r0e[:])
            a1 = newt()
            nc.vector.tensor_mul(a1[:], a0[:], s0[:])
            a2 = newt()
            nc.vector.tensor_scalar(a2[:], a1[:], -0.5, 1.5, OP.mult, OP.add)
            ri = newt()
            nc.vector.tensor_mul(ri[:], r0e[:], a2[:])
            normS = newt()
            nc.vector.tensor_mul(normS[:], s0[:], ri[:])
            mask = newt()
            nc.vector.tensor_scalar(mask[:], sumsq[:], 1e-12, None, OP.is_gt)
            sm = newt()
            nc.vector.tensor_mul(sm[:], colsum[:], ri[:])
            S = newt()
            nc.vector.tensor_mul(S[:], sm[:], mask[:])
            l1 = newt()
            nc.vector.tensor_scalar_mul(l1[:], cnt[:], K_CONST)
            l2 = newt()
            nc.vector.tensor_scalar_mul(l2[:], colsum[:], INV_D5)
            lseg = newt()
            nc.vector.tensor_add(lseg[:], l1[:], l2[:])
            aa = newt()
            nc.vector.tensor_mul(aa[:], S[:], lseg[:])
            bb = newt()
            nc.vector.tensor_mul(bb[:], normS[:], mask[:])
            nbb = newt()
            nc.vector.tensor_scalar_mul(nbb[:], bb[:], -1.0)
            num = newt()
            nc.vector.tensor_add(num[:], aa[:], nbb[:])
            cc = newt()
            nc.vector.tensor_scalar_max(cc[:], cnt[:], 1.0)
            ic = newt()
            nc.vector.reciprocal(ic[:], cc[:])
            loss = newt()
            nc.vector.tensor_mul(loss[:], num[:], ic[:])

            nc.sync.dma_start(out=out_ext[:, :], in_=loss[:])

    nc.compile()
    return nc


def assign_classes(labels):
    """Greedy balanced partition: 128 classes per core, near-equal row totals.
    Returns (owner_of_cls [C], pos_of_cls [C], cls_at [NCORES, CLOC])."""
    counts = np.bincount(labels, minlength=C)
    order = np.argsort(-counts, kind="stable")
    bin_rows = np.zeros(NCORES, dtype=np.int64)
    bin_n = np.zeros(NCORES, dtype=np.int64)
    owner_of_cls = np.empty(C, dtype=np.int64)
    pos_of_cls = np.empty(C, dtype=np.int64)
    cls_at = np.empty((NCORES, CLOC), dtype=np.int64)
    for cidx in order:
        open_bins = np.flatnonzero(bin_n < CLOC)
        k = open_bins[np.argmin(bin_rows[open_bins])]
        owner_of_cls[cidx] = k
        pos_of_cls[cidx] = bin_n[k]
        cls_at[k, bin_n[k]] = cidx
        bin_n[k] += 1
        bin_rows[k] += counts[cidx]
    return owner_of_cls, pos_of_cls, cls_at, bin_rows


def make_in_maps(logits, labels):
    """Host-side sharding: route each row to the core owning its (balanced)
    class bin; lay X out so each partition's per-group data is contiguous."""
    logits = np.ascontiguousarray(np.asarray(logits, dtype=np.float32))
    labels = np.asarray(labels).astype(np.int64)
    owner_of_cls, pos_of_cls, cls_at, bin_rows = assign_classes(labels)
    assert bin_rows.max() <= CAP, f"max shard {bin_rows.max()} > capacity {CAP}"
    owner = owner_of_cls[labels]
    local = pos_of_cls[labels]
    in_maps = []
    iota_tile = np.ascontiguousarray(
        np.broadcast_to(np.arange(CLOC, dtype=np.float32), (P, CLOC))
    )
    for k in range(NCORES):
        idx = np.flatnonzero(owner == k)
        nk = idx.size
        xs = np.zeros((CAP, D), dtype=np.float32)
        xs[:nk] = logits[idx]
        # full groups: row (g*G + j)*P + p -> x4[g, p, j, :]
        x4 = np.ascontiguousarray(
            xs[: NG * G * P].reshape(NG, G, P, D).transpose(0, 2, 1, 3)
        )
        xt = np.ascontiguousarray(
            xs[NG * G * P :].reshape(G_TAIL, P, D).transpose(1, 0, 2)
        )
        ll = np.full((CAP,), -1.0, dtype=np.float32)
        ll[:nk] = local[idx].astype(np.float32)
        lab2d = np.ascontiguousarray(ll.reshape(NT, P).T)  # [p, t] = ll[t*128+p]
        in_maps.append(
            {"x": x4, "xt": xt, "lab": lab2d, "iota": iota_tile}
        )
    return in_maps, cls_at


_NC_CACHE = {}


def get_nc():
    if "nc" not in _NC_CACHE:
        _NC_CACHE["nc"] = build_nc()
    return _NC_CACHE["nc"]


def run(logits, labels, num_classes, trace=False, **spmd_kwargs):
    assert int(num_classes) == C
    nc = get_nc()
    in_maps, cls_at = make_in_maps(logits, labels)
    res = run_bass_kernel_spmd(
        nc, in_maps, core_ids=list(range(NCORES)), trace=trace, **spmd_kwargs
    )
    out = np.empty((C,), dtype=np.float32)
    for k in range(NCORES):
        out[cls_at[k]] = res.results[k]["out"].ravel()
    return out, res


def kernel(logits, labels, num_classes):
    out, _ = run(logits, labels, num_classes)
    return out


# revision 22
# speedup vs baseline: 1.8851x; 1.8851x over previous
"""ArcFace-style per-class loss kernel for 8 Trainium2 NeuronCores.

Math (algebraically exact reduction of the reference):
  Xn_i  = X_i / ||X_i||
  sums_c = sum_{i: l_i=c} Xn_i               [C, D] segment sum
  counts_c = |{i: l_i=c}|
  loss_c = (S_c * lse_seg_c - ||sums_c||) / max(counts_c, 1)
    with S_c = colsum_c/||sums_c||, colsum_c = sum_d sums_c[d]
  Because rows are unit-norm, lse_i = log(D + 1/2 + sum_d Xn_id) + O(1e-5)
  (2nd-order Taylor of logsumexp using sum_d Xn^2 = 1), so
  lse_seg_c = K*counts_c + colsum_c/(D+1/2),  K = log(D+1/2).

Sharding: rows are routed (on host) to the core owning their label octant
(core k owns classes [128k, 128k+128)), so every per-class reduction is
fully local to one core — no collectives.  Host also lays X out so each
partition's group data is contiguous in DRAM (16 KB reads).

Per 128-row tile: row sum-of-squares on ACT (Square+accumulate) or DVE
(scalar_tensor_tensor), balanced so both engines stay under the DMA
budget; rsqrt via sqrt+reciprocal+Newton (batched per group); scaled
one-hot = (iota==label)*rnorm in one fused DVE tensor_scalar; bf16 cast of
X is one group-wide DVE copy; PE accumulates sums (one-hotT @ Xbf) and
compensated counts (one-hotT @ (ss*rnorm)) into PSUM across all tiles.
Padded rows have label -1 (zero one-hot column) and X = 0.
"""

import sys

if "/opt/trn_rl_repo" not in sys.path:
    sys.path.insert(0, "/opt/trn_rl_repo")

import math

import numpy as np

import concourse.bass as bass  # noqa: F401
import concourse.tile as tile
from concourse import bacc, mybir
from concourse.bass_utils import run_bass_kernel_spmd

# Problem constants (hardcoded per spec: N=131072, D=512, C=1024, 8 cores)
N_ROWS = 131072
D = 512
C = 1024
NCORES = 8
CLOC = C // NCORES  # 128 classes per core

# Classes are assigned to cores by balanced greedy bin-packing (128 classes
# per core, near-equal row totals), so per-core rows ~ N/8 = 16384 +- ~16.
# Capacity 16640 = 16 full groups of 1024 rows + one 2-tile tail group.
CAP = 16640
P = 128  # partitions / rows per tile
NT = CAP // P  # 130 tiles
G = 8  # tiles per full group (one DMA per group)
NG = 16  # full groups
G_TAIL = 2  # tiles in the tail group

K_CONST = math.log(D + 0.5)
INV_D5 = 1.0 / (D + 0.5)

F32 = mybir.dt.float32
BF16 = mybir.dt.bfloat16


def build_nc():
    nc = bacc.Bacc(None, target_bir_lowering=False)

    x_ext = nc.declare_dram_parameter("x", [NG, P, G, D], F32, isOutput=False)
    xt_ext = nc.declare_dram_parameter("xt", [P, G_TAIL, D], F32, isOutput=False)
    lab_ext = nc.declare_dram_parameter("lab", [P, NT], F32, isOutput=False)
    iota_ext = nc.declare_dram_parameter("iota", [P, CLOC], F32, isOutput=False)
    out_ext = nc.declare_dram_parameter("out", [P, 1], F32, isOutput=True)

    AF = mybir.ActivationFunctionType
    OP = mybir.AluOpType

    with tile.TileContext(nc) as tc:
        with (
            tc.tile_pool(name="xpool", bufs=4) as xpool,
            tc.tile_pool(name="ohpool", bufs=8) as ohpool,
            tc.tile_pool(name="small", bufs=6) as small,
            tc.tile_pool(name="singles", bufs=1) as singles,
            tc.tile_pool(name="psum", bufs=1, space="PSUM") as psum,
        ):
            lab_sb = singles.tile([P, NT], F32)
            nc.sync.dma_start(out=lab_sb[:], in_=lab_ext[:, :])
            iota_sb = singles.tile([P, CLOC], F32)
            nc.sync.dma_start(out=iota_sb[:], in_=iota_ext[:, :])

            # prefetch the sqrt activation table while the first DMAs run
            warm = singles.tile([P, 1], F32)
            nc.vector.memset(warm[:], 1.0)
            nc.scalar.activation(out=warm[:], in_=warm[:], func=AF.Sqrt)

            psum_sums = psum.tile([P, D], F32)  # one full bank
            psum_cnt = psum.tile([P, 1], F32)
            act_scratch = psum.tile([P, D], F32)  # ACT Square dump
            dve_scratch = singles.tile([P, D], F32)  # DVE stt dump

            def process_group(g, t_base, src_ap, gg, n_dve):
                xg = xpool.tile([P, gg, D], F32, tag="xg", name=f"xg{g}")
                nc.sync.dma_start(out=xg[:], in_=src_ap)

                xbf = xpool.tile([P, gg, D], BF16, tag="xbf", name=f"xbf{g}")
                nc.vector.tensor_copy(xbf[:], xg[:])

                # per-row sum of squares, split ACT / DVE to balance load
                ssg = small.tile([P, gg], F32, tag="ssg", name=f"ssg{g}")
                for j in range(gg):
                    if j >= gg - n_dve:
                        nc.vector.scalar_tensor_tensor(
                            out=dve_scratch[:],
                            in0=xg[:, j],
                            scalar=1.0,
                            in1=xg[:, j],
                            op0=OP.mult,
                            op1=OP.mult,
                            accum_out=ssg[:, j : j + 1],
                        )
                    else:
                        nc.scalar.activation(
                            out=act_scratch[:],
                            in_=xg[:, j],
                            func=AF.Square,
                            accum_out=ssg[:, j : j + 1],
                        )

                # rnorm = 1/sqrt(max(ss, eps)), Newton-refined; ncol = ss*rnorm
                def st(nm):
                    return small.tile([P, gg], F32, tag=nm, name=f"{nm}{g}")

                ssc = st("ssc")
                nc.vector.tensor_scalar_max(ssc[:], ssg[:], 1e-12)
                sqg = st("sqg")
                nc.scalar.activation(out=sqg[:], in_=ssc[:], func=AF.Sqrt)
                r0 = st("r0")
                nc.vector.reciprocal(r0[:], sqg[:])
                t0 = st("t0")
                nc.vector.tensor_mul(t0[:], r0[:], r0[:])
                t1 = st("t1")
                nc.vector.tensor_mul(t1[:], t0[:], ssc[:])
                t2 = st("t2")
                nc.vector.tensor_scalar(t2[:], t1[:], -0.5, 1.5, OP.mult, OP.add)
                rn = st("rn")
                nc.vector.tensor_mul(rn[:], r0[:], t2[:])
                ncol = st("ncol")
                nc.vector.tensor_mul(ncol[:], ssc[:], rn[:])
                ncbf = small.tile([P, gg], BF16, tag="ncbf", name=f"ncbf{g}")
                nc.vector.tensor_copy(ncbf[:], ncol[:])

                for j in range(gg):
                    t = t_base + j
                    oh = ohpool.tile([P, CLOC], BF16, tag="oh", name=f"oh{t}")
                    nc.vector.tensor_scalar(
                        oh[:],
                        iota_sb[:],
                        lab_sb[:, t : t + 1],
                        rn[:, j : j + 1],
                        OP.is_equal,
                        OP.mult,
                    )
                    nc.tensor.matmul(
                        psum_sums[:],
                        lhsT=oh[:],
                        rhs=xbf[:, j],
                        start=(t == 0),
                        stop=(t == NT - 1),
                    )
                    nc.tensor.matmul(
                        psum_cnt[:],
                        lhsT=oh[:],
                        rhs=ncbf[:, j : j + 1],
                        start=(t == 0),
                        stop=(t == NT - 1),
                    )

            for g in range(NG):
                process_group(
                    g, g * G, x_ext[g], G, n_dve=(1 if (g % 2 == 0) else 2)
                )
            process_group(NG, NG * G, xt_ext[:, :, :], G_TAIL, n_dve=1)

            # ---- epilogue: per-class loss from sums/counts ----
            sums_sb = singles.tile([P, D], F32)
            nc.vector.tensor_copy(sums_sb[:], psum_sums[:])
            cnt = singles.tile([P, 1], F32)
            nc.vector.tensor_copy(cnt[:], psum_cnt[:])

            colsum = singles.tile([P, 1], F32)
            nc.vector.tensor_reduce(
                colsum[:], sums_sb[:], mybir.AxisListType.X, OP.add
            )
            junk = singles.tile([P, D], F32)
            nc.vector.tensor_mul(junk[:], sums_sb[:], sums_sb[:])
            sumsq = singles.tile([P, 1], F32)
            nc.vector.tensor_reduce(
                sumsq[:], junk[:], mybir.AxisListType.X, OP.add
            )

            _ep_n = [0]

            def newt():
                _ep_n[0] += 1
                return singles.tile(
                    [P, 1], F32, name=f"ep{_ep_n[0]}", tag=f"ep{_ep_n[0]}"
                )

            s0 = newt()
            nc.vector.tensor_scalar_max(s0[:], sumsq[:], 1e-20)
            sq2 = newt()
            nc.scalar.activation(out=sq2[:], in_=s0[:], func=AF.Sqrt)
            r0e = newt()
            nc.vector.reciprocal(r0e[:], sq2[:])
            a0 = newt()
            nc.vector.tensor_mul(a0[:], r0e[:], r0e[:])
            a1 = newt()
            nc.vector.tensor_mul(a1[:], a0[:], s0[:])
            a2 = newt()
            nc.vector.tensor_scalar(a2[:], a1[:], -0.5, 1.5, OP.mult, OP.add)
            ri = newt()
            nc.vector.tensor_mul(ri[:], r0e[:], a2[:])
            normS = newt()
            nc.vector.tensor_mul(normS[:], s0[:], ri[:])
            mask = newt()
            nc.vector.tensor_scalar(mask[:], sumsq[:], 1e-12, None, OP.is_gt)
            sm = newt()
            nc.vector.tensor_mul(sm[:], colsum[:], ri[:])
            S = newt()
            nc.vector.tensor_mul(S[:], sm[:], mask[:])
            l1 = newt()
            nc.vector.tensor_scalar_mul(l1[:], cnt[:], K_CONST)
            l2 = newt()
            nc.vector.tensor_scalar_mul(l2[:], colsum[:], INV_D5)
            lseg = newt()
            nc.vector.tensor_add(lseg[:], l1[:], l2[:])
            aa = newt()
            nc.vector.tensor_mul(aa[:], S[:], lseg[:])
            bb = newt()
            nc.vector.tensor_mul(bb[:], normS[:], mask[:])
            nbb = newt()
            nc.vector.tensor_scalar_mul(nbb[:], bb[:], -1.0)
            num = newt()
            nc.vector.tensor_add(num[:], aa[:], nbb[:])
            cc = newt()
            nc.vector.tensor_scalar_max(cc[:], cnt[:], 1.0)
            ic = newt()
            nc.vector.reciprocal(ic[:], cc[:])
            loss = newt()
            nc.vector.tensor_mul(loss[:], num[:], ic[:])

            nc.sync.dma_start(out=out_ext[:, :], in_=loss[:])

    nc.compile()
    return nc


def assign_classes(labels):
    """Greedy balanced partition: 128 classes per core, near-equal row totals.
    Returns (owner_of_cls [C], pos_of_cls [C], cls_at [NCORES, CLOC])."""
    counts = np.bincount(labels, minlength=C)
    order = np.argsort(-counts, kind="stable")
    bin_rows = np.zeros(NCORES, dtype=np.int64)
    bin_n = np.zeros(NCORES, dtype=np.int64)
    owner_of_cls = np.empty(C, dtype=np.int64)
    pos_of_cls = np.empty(C, dtype=np.int64)
    cls_at = np.empty((NCORES, CLOC), dtype=np.int64)
    for cidx in order:
        open_bins = np.flatnonzero(bin_n < CLOC)
        k = open_bins[np.argmin(bin_rows[open_bins])]
        owner_of_cls[cidx] = k
        pos_of_cls[cidx] = bin_n[k]
        cls_at[k, bin_n[k]] = cidx
        bin_n[k] += 1
        bin_rows[k] += counts[cidx]
    return owner_of_cls, pos_of_cls, cls_at, bin_rows


def make_in_maps(logits, labels):
    """Host-side sharding: route each row to the core owning its (balanced)
    class bin; lay X out so each partition's per-group data is contiguous."""
    logits = np.ascontiguousarray(np.asarray(logits, dtype=np.float32))
    labels = np.asarray(labels).astype(np.int64)
    owner_of_cls, pos_of_cls, cls_at, bin_rows = assign_classes(labels)
    assert bin_rows.max() <= CAP, f"max shard {bin_rows.max()} > capacity {CAP}"
    owner = owner_of_cls[labels]
    local = pos_of_cls[labels]
    in_maps = []
    iota_tile = np.ascontiguousarray(
        np.broadcast_to(np.arange(CLOC, dtype=np.float32), (P, CLOC))
    )
    for k in range(NCORES):
        idx = np.flatnonzero(owner == k)
        nk = idx.size
        xs = np.zeros((CAP, D), dtype=np.float32)
        xs[:nk] = logits[idx]
        # full groups: row (g*G + j)*P + p -> x4[g, p, j, :]
        x4 = np.ascontiguousarray(
            xs[: NG * G * P].reshape(NG, G, P, D).transpose(0, 2, 1, 3)
        )
        xt = np.ascontiguousarray(
            xs[NG * G * P :].reshape(G_TAIL, P, D).transpose(1, 0, 2)
        )
        ll = np.full((CAP,), -1.0, dtype=np.float32)
        ll[:nk] = local[idx].astype(np.float32)
        lab2d = np.ascontiguousarray(ll.reshape(NT, P).T)  # [p, t] = ll[t*128+p]
        in_maps.append(
            {"x": x4, "xt": xt, "lab": lab2d, "iota": iota_tile}
        )
    return in_maps, cls_at


_NC_CACHE = {}


def get_nc():
    if "nc" not in _NC_CACHE:
        _NC_CACHE["nc"] = build_nc()
    return _NC_CACHE["nc"]


def run(logits, labels, num_classes, trace=False, **spmd_kwargs):
    assert int(num_classes) == C
    nc = get_nc()
    in_maps, cls_at = make_in_maps(logits, labels)
    res = run_bass_kernel_spmd(
        nc, in_maps, core_ids=list(range(NCORES)), trace=trace, **spmd_kwargs
    )
    out = np.empty((C,), dtype=np.float32)
    for k in range(NCORES):
        out[cls_at[k]] = res.results[k]["out"].ravel()
    return out, res


def kernel(logits, labels, num_classes):
    out, _ = run(logits, labels, num_classes)
    return out


# revision 24
# speedup vs baseline: 1.8930x; 1.0042x over previous
"""ArcFace-style per-class loss kernel for 8 Trainium2 NeuronCores.

Math (algebraically exact reduction of the reference):
  Xn_i  = X_i / ||X_i||
  sums_c = sum_{i: l_i=c} Xn_i               [C, D] segment sum
  counts_c = |{i: l_i=c}|
  loss_c = (S_c * lse_seg_c - ||sums_c||) / max(counts_c, 1)
    with S_c = colsum_c/||sums_c||, colsum_c = sum_d sums_c[d]
  Because rows are unit-norm, lse_i = log(D + 1/2 + sum_d Xn_id) + O(1e-5)
  (2nd-order Taylor of logsumexp using sum_d Xn^2 = 1), so
  lse_seg_c = K*counts_c + colsum_c/(D+1/2),  K = log(D+1/2).

Sharding: rows are routed (on host) to the core owning their label octant
(core k owns classes [128k, 128k+128)), so every per-class reduction is
fully local to one core — no collectives.  Host also lays X out so each
partition's group data is contiguous in DRAM (16 KB reads).

Per 128-row tile: row sum-of-squares on ACT (Square+accumulate) or DVE
(scalar_tensor_tensor), balanced so both engines stay under the DMA
budget; rsqrt via sqrt+reciprocal+Newton (batched per group); scaled
one-hot = (iota==label)*rnorm in one fused DVE tensor_scalar; bf16 cast of
X is one group-wide DVE copy; PE accumulates sums (one-hotT @ Xbf) and
compensated counts (one-hotT @ (ss*rnorm)) into PSUM across all tiles.
Padded rows have label -1 (zero one-hot column) and X = 0.
"""

import sys

if "/opt/trn_rl_repo" not in sys.path:
    sys.path.insert(0, "/opt/trn_rl_repo")

import math

import numpy as np

import concourse.bass as bass  # noqa: F401
import concourse.tile as tile
from concourse import bacc, mybir
from concourse.bass_utils import run_bass_kernel_spmd

# Problem constants (hardcoded per spec: N=131072, D=512, C=1024, 8 cores)
N_ROWS = 131072
D = 512
C = 1024
NCORES = 8
CLOC = C // NCORES  # 128 classes per core

# Classes are assigned to cores by balanced greedy bin-packing (128 classes
# per core, near-equal row totals), so per-core rows ~ N/8 = 16384 +- ~16.
# Capacity 16640 = 16 full groups of 1024 rows + one 2-tile tail group.
CAP = 16640
P = 128  # partitions / rows per tile
NT = CAP // P  # 130 tiles
G = 8  # tiles per full group (one DMA per group)
NG = 16  # full groups
G_TAIL = 2  # tiles in the tail group

K_CONST = math.log(D + 0.5)
INV_D5 = 1.0 / (D + 0.5)

F32 = mybir.dt.float32
BF16 = mybir.dt.bfloat16


def build_nc():
    nc = bacc.Bacc(None, target_bir_lowering=False)

    x_ext = nc.declare_dram_parameter("x", [NG, P, G, D], F32, isOutput=False)
    xt_ext = nc.declare_dram_parameter("xt", [P, G_TAIL, D], F32, isOutput=False)
    lab_ext = nc.declare_dram_parameter("lab", [P, NT], F32, isOutput=False)
    iota_ext = nc.declare_dram_parameter("iota", [P, CLOC], F32, isOutput=False)
    out_ext = nc.declare_dram_parameter("out", [P, 1], F32, isOutput=True)

    AF = mybir.ActivationFunctionType
    OP = mybir.AluOpType

    with tile.TileContext(nc) as tc:
        with (
            tc.tile_pool(name="xpool", bufs=4) as xpool,
            tc.tile_pool(name="ohpool", bufs=8) as ohpool,
            tc.tile_pool(name="small", bufs=6) as small,
            tc.tile_pool(name="singles", bufs=1) as singles,
            tc.tile_pool(name="psum", bufs=1, space="PSUM") as psum,
        ):
            lab_sb = singles.tile([P, NT], F32)
            nc.sync.dma_start(out=lab_sb[:], in_=lab_ext[:, :])
            iota_sb = singles.tile([P, CLOC], F32)
            nc.sync.dma_start(out=iota_sb[:], in_=iota_ext[:, :])

            # prefetch the sqrt activation table while the first DMAs run
            warm = singles.tile([P, 1], F32)
            nc.vector.memset(warm[:], 1.0)
            nc.scalar.activation(out=warm[:], in_=warm[:], func=AF.Sqrt)

            psum_sums = psum.tile([P, D], F32)  # one full bank
            psum_cnt = psum.tile([P, 1], F32)
            act_scratch = psum.tile([P, D], F32)  # ACT Square dump
            dve_scratch_bf = singles.tile([P, D], BF16)  # DVE stt dump

            def process_group(g, t_base, src_ap, gg, n_dve):
                xg = xpool.tile([P, gg, D], F32, tag="xg", name=f"xg{g}")
                nc.sync.dma_start(out=xg[:], in_=src_ap)

                xbf = xpool.tile([P, gg, D], BF16, tag="xbf", name=f"xbf{g}")
                nc.vector.tensor_copy(xbf[:], xg[:])

                # per-row sum of squares, split ACT / DVE to balance load
                ssg = small.tile([P, gg], F32, tag="ssg", name=f"ssg{g}")
                for j in range(gg):
                    if j >= gg - n_dve:
                        # squares from the bf16 copy: 16-bit inputs are
                        # eligible for the DVE 2x mode; precision impact on
                        # ||X|| is ~0.04%, far below the bf16 matmul noise.
                        nc.vector.scalar_tensor_tensor(
                            out=dve_scratch_bf[:],
                            in0=xbf[:, j],
                            scalar=1.0,
                            in1=xbf[:, j],
                            op0=OP.mult,
                            op1=OP.mult,
                            accum_out=ssg[:, j : j + 1],
                        )
                    else:
                        nc.scalar.activation(
                            out=act_scratch[:],
                            in_=xg[:, j],
                            func=AF.Square,
                            accum_out=ssg[:, j : j + 1],
                        )

                # rnorm = 1/sqrt(max(ss, eps)), Newton-refined; ncol = ss*rnorm
                def st(nm):
                    return small.tile([P, gg], F32, tag=nm, name=f"{nm}{g}")

                ssc = st("ssc")
                nc.vector.tensor_scalar_max(ssc[:], ssg[:], 1e-12)
                sqg = st("sqg")
                nc.scalar.activation(out=sqg[:], in_=ssc[:], func=AF.Sqrt)
                r0 = st("r0")
                nc.vector.reciprocal(r0[:], sqg[:])
                t0 = st("t0")
                nc.vector.tensor_mul(t0[:], r0[:], r0[:])
                t1 = st("t1")
                nc.vector.tensor_mul(t1[:], t0[:], ssc[:])
                t2 = st("t2")
                nc.vector.tensor_scalar(t2[:], t1[:], -0.5, 1.5, OP.mult, OP.add)
                rn = st("rn")
                nc.vector.tensor_mul(rn[:], r0[:], t2[:])
                ncol = st("ncol")
                nc.vector.tensor_mul(ncol[:], ssc[:], rn[:])
                ncbf = small.tile([P, gg], BF16, tag="ncbf", name=f"ncbf{g}")
                nc.vector.tensor_copy(ncbf[:], ncol[:])

                for j in range(gg):
                    t = t_base + j
                    oh = ohpool.tile([P, CLOC], BF16, tag="oh", name=f"oh{t}")
                    nc.vector.tensor_scalar(
                        oh[:],
                        iota_sb[:],
                        lab_sb[:, t : t + 1],
                        rn[:, j : j + 1],
                        OP.is_equal,
                        OP.mult,
                    )
                    nc.tensor.matmul(
                        psum_sums[:],
                        lhsT=oh[:],
                        rhs=xbf[:, j],
                        start=(t == 0),
                        stop=(t == NT - 1),
                    )
                    nc.tensor.matmul(
                        psum_cnt[:],
                        lhsT=oh[:],
                        rhs=ncbf[:, j : j + 1],
                        start=(t == 0),
                        stop=(t == NT - 1),
                    )

            for g in range(NG):
                process_group(
                    g, g * G, x_ext[g], G, n_dve=(1 if (g % 2 == 0) else 2)
                )
            process_group(NG, NG * G, xt_ext[:, :, :], G_TAIL, n_dve=1)

            # ---- epilogue: per-class loss from sums/counts ----
            sums_sb = singles.tile([P, D], F32)
            nc.vector.tensor_copy(sums_sb[:], psum_sums[:])
            cnt = singles.tile([P, 1], F32)
            nc.vector.tensor_copy(cnt[:], psum_cnt[:])

            colsum = singles.tile([P, 1], F32)
            nc.vector.tensor_reduce(
                colsum[:], sums_sb[:], mybir.AxisListType.X, OP.add
            )
            junk = singles.tile([P, D], F32)
            nc.vector.tensor_mul(junk[:], sums_sb[:], sums_sb[:])
            sumsq = singles.tile([P, 1], F32)
            nc.vector.tensor_reduce(
                sumsq[:], junk[:], mybir.AxisListType.X, OP.add
            )

            _ep_n = [0]

            def newt():
                _ep_n[0] += 1
                return singles.tile(
                    [P, 1], F32, name=f"ep{_ep_n[0]}", tag=f"ep{_ep_n[0]}"
                )

            s0 = newt()
            nc.vector.tensor_scalar_max(s0[:], sumsq[:], 1e-20)
            sq2 = newt()
            nc.scalar.activation(out=sq2[:], in_=s0[:], func=AF.Sqrt)
            r0e = newt()
            nc.vector.reciprocal(r0e[:], sq2[:])
            a0 = newt()
            nc.vector.tensor_mul(a0[:], r0e[:], r0e[:])
            a1 = newt()
            nc.vector.tensor_mul(a1[:], a0[:], s0[:])
            a2 = newt()
            nc.vector.tensor_scalar(a2[:], a1[:], -0.5, 1.5, OP.mult, OP.add)
            ri = newt()
            nc.vector.tensor_mul(ri[:], r0e[:], a2[:])
            normS = newt()
            nc.vector.tensor_mul(normS[:], s0[:], ri[:])
            mask = newt()
            nc.vector.tensor_scalar(mask[:], sumsq[:], 1e-12, None, OP.is_gt)
            sm = newt()
            nc.vector.tensor_mul(sm[:], colsum[:], ri[:])
            S = newt()
            nc.vector.tensor_mul(S[:], sm[:], mask[:])
            l1 = newt()
            nc.vector.tensor_scalar_mul(l1[:], cnt[:], K_CONST)
            l2 = newt()
            nc.vector.tensor_scalar_mul(l2[:], colsum[:], INV_D5)
            lseg = newt()
            nc.vector.tensor_add(lseg[:], l1[:], l2[:])
            aa = newt()
            nc.vector.tensor_mul(aa[:], S[:], lseg[:])
            bb = newt()
            nc.vector.tensor_mul(bb[:], normS[:], mask[:])
            nbb = newt()
            nc.vector.tensor_scalar_mul(nbb[:], bb[:], -1.0)
            num = newt()
            nc.vector.tensor_add(num[:], aa[:], nbb[:])
            cc = newt()
            nc.vector.tensor_scalar_max(cc[:], cnt[:], 1.0)
            ic = newt()
            nc.vector.reciprocal(ic[:], cc[:])
            loss = newt()
            nc.vector.tensor_mul(loss[:], num[:], ic[:])

            nc.sync.dma_start(out=out_ext[:, :], in_=loss[:])

    nc.compile()
    return nc


def assign_classes(labels):
    """Greedy balanced partition: 128 classes per core, near-equal row totals.
    Returns (owner_of_cls [C], pos_of_cls [C], cls_at [NCORES, CLOC])."""
    counts = np.bincount(labels, minlength=C)
    order = np.argsort(-counts, kind="stable")
    bin_rows = np.zeros(NCORES, dtype=np.int64)
    bin_n = np.zeros(NCORES, dtype=np.int64)
    owner_of_cls = np.empty(C, dtype=np.int64)
    pos_of_cls = np.empty(C, dtype=np.int64)
    cls_at = np.empty((NCORES, CLOC), dtype=np.int64)
    for cidx in order:
        open_bins = np.flatnonzero(bin_n < CLOC)
        k = open_bins[np.argmin(bin_rows[open_bins])]
        owner_of_cls[cidx] = k
        pos_of_cls[cidx] = bin_n[k]
        cls_at[k, bin_n[k]] = cidx
        bin_n[k] += 1
        bin_rows[k] += counts[cidx]
    return owner_of_cls, pos_of_cls, cls_at, bin_rows


def make_in_maps(logits, labels):
    """Host-side sharding: route each row to the core owning its (balanced)
    class bin; lay X out so each partition's per-group data is contiguous."""
    logits = np.ascontiguousarray(np.asarray(logits, dtype=np.float32))
    labels = np.asarray(labels).astype(np.int64)
    owner_of_cls, pos_of_cls, cls_at, bin_rows = assign_classes(labels)
    assert bin_rows.max() <= CAP, f"max shard {bin_rows.max()} > capacity {CAP}"
    owner = owner_of_cls[labels]
    local = pos_of_cls[labels]
    in_maps = []
    iota_tile = np.ascontiguousarray(
        np.broadcast_to(np.arange(CLOC, dtype=np.float32), (P, CLOC))
    )
    for k in range(NCORES):
        idx = np.flatnonzero(owner == k)
        nk = idx.size
        xs = np.zeros((CAP, D), dtype=np.float32)
        xs[:nk] = logits[idx]
        # full groups: row (g*G + j)*P + p -> x4[g, p, j, :]
        x4 = np.ascontiguousarray(
            xs[: NG * G * P].reshape(NG, G, P, D).transpose(0, 2, 1, 3)
        )
        xt = np.ascontiguousarray(
            xs[NG * G * P :].reshape(G_TAIL, P, D).transpose(1, 0, 2)
        )
        ll = np.full((CAP,), -1.0, dtype=np.float32)
        ll[:nk] = local[idx].astype(np.float32)
        lab2d = np.ascontiguousarray(ll.reshape(NT, P).T)  # [p, t] = ll[t*128+p]
        in_maps.append(
            {"x": x4, "xt": xt, "lab": lab2d, "iota": iota_tile}
        )
    return in_maps, cls_at


_NC_CACHE = {}


def get_nc():
    if "nc" not in _NC_CACHE:
        _NC_CACHE["nc"] = build_nc()
    return _NC_CACHE["nc"]


def run(logits, labels, num_classes, trace=False, **spmd_kwargs):
    assert int(num_classes) == C
    nc = get_nc()
    in_maps, cls_at = make_in_maps(logits, labels)
    res = run_bass_kernel_spmd(
        nc, in_maps, core_ids=list(range(NCORES)), trace=trace, **spmd_kwargs
    )
    out = np.empty((C,), dtype=np.float32)
    for k in range(NCORES):
        out[cls_at[k]] = res.results[k]["out"].ravel()
    return out, res


def kernel(logits, labels, num_classes):
    out, _ = run(logits, labels, num_classes)
    return out


# revision 28
# speedup vs baseline: 2.2987x; 1.2143x over previous
"""ArcFace-style per-class loss kernel for 8 Trainium2 NeuronCores.

Math (algebraically exact reduction of the reference):
  Xn_i  = X_i / ||X_i||
  sums_c = sum_{i: l_i=c} Xn_i               [C, D] segment sum
  counts_c = |{i: l_i=c}|
  loss_c = (S_c * lse_seg_c - ||sums_c||) / max(counts_c, 1)
    with S_c = colsum_c/||sums_c||, colsum_c = sum_d sums_c[d]
  Because rows are unit-norm, lse_i = log(D + 1/2 + sum_d Xn_id) + O(1e-5)
  (2nd-order Taylor of logsumexp using sum_d Xn^2 = 1), so
  lse_seg_c = K*counts_c + colsum_c/(D+1/2),  K = log(D+1/2).

Sharding: rows are routed (on host) to the core owning their label octant
(core k owns classes [128k, 128k+128)), so every per-class reduction is
fully local to one core — no collectives.  Host also lays X out so each
partition's group data is contiguous in DRAM (16 KB reads).

Per 128-row tile: row sum-of-squares on ACT (Square+accumulate) or DVE
(scalar_tensor_tensor), balanced so both engines stay under the DMA
budget; rsqrt via sqrt+reciprocal+Newton (batched per group); scaled
one-hot = (iota==label)*rnorm in one fused DVE tensor_scalar; bf16 cast of
X is one group-wide DVE copy; PE accumulates sums (one-hotT @ Xbf) and
compensated counts (one-hotT @ (ss*rnorm)) into PSUM across all tiles.
Padded rows have label -1 (zero one-hot column) and X = 0.
"""

import sys

if "/opt/trn_rl_repo" not in sys.path:
    sys.path.insert(0, "/opt/trn_rl_repo")

import math

import ml_dtypes
import numpy as np

import concourse.bass as bass  # noqa: F401
import concourse.tile as tile
from concourse import bacc, mybir
from concourse.bass_utils import run_bass_kernel_spmd

# Problem constants (hardcoded per spec: N=131072, D=512, C=1024, 8 cores)
N_ROWS = 131072
D = 512
C = 1024
NCORES = 8
CLOC = C // NCORES  # 128 classes per core

# Classes are assigned to cores by balanced greedy bin-packing (128 classes
# per core, near-equal row totals), so per-core rows ~ N/8 = 16384 +- ~16.
# Capacity 16640 = 16 full groups of 1024 rows + one 2-tile tail group.
CAP = 16640
P = 128  # partitions / rows per tile
NT = CAP // P  # 130 tiles
G = 8  # tiles per full group (one DMA per group)
NG = 16  # full groups
G_TAIL = 2  # tiles in the tail group

K_CONST = math.log(D + 0.5)
INV_D5 = 1.0 / (D + 0.5)

F32 = mybir.dt.float32
BF16 = mybir.dt.bfloat16


def build_nc():
    nc = bacc.Bacc(None, target_bir_lowering=False)

    x_ext = nc.declare_dram_parameter("x", [NG, P, G, D], F32, isOutput=False)
    xt_ext = nc.declare_dram_parameter("xt", [P, G_TAIL, D], F32, isOutput=False)
    lab_ext = nc.declare_dram_parameter("lab", [P, NT], F32, isOutput=False)
    iota_ext = nc.declare_dram_parameter("iota", [P, CLOC], BF16, isOutput=False)
    out_ext = nc.declare_dram_parameter("out", [P, 1], F32, isOutput=True)

    AF = mybir.ActivationFunctionType
    OP = mybir.AluOpType

    with tile.TileContext(nc) as tc:
        with (
            tc.tile_pool(name="xpool", bufs=4) as xpool,
            tc.tile_pool(name="ohpool", bufs=8) as ohpool,
            tc.tile_pool(name="small", bufs=6) as small,
            tc.tile_pool(name="singles", bufs=1) as singles,
            tc.tile_pool(name="psum", bufs=1, space="PSUM") as psum,
        ):
            lab_sb = singles.tile([P, NT], F32)
            nc.sync.dma_start(out=lab_sb[:], in_=lab_ext[:, :])
            iota_sb = singles.tile([P, CLOC], BF16)
            nc.sync.dma_start(out=iota_sb[:], in_=iota_ext[:, :])

            # prefetch the sqrt activation table while the first DMAs run
            warm = singles.tile([P, 1], F32)
            nc.vector.memset(warm[:], 1.0)
            nc.scalar.activation(out=warm[:], in_=warm[:], func=AF.Sqrt)

            psum_sums = psum.tile([P, D], F32)  # one full bank
            psum_cnt = psum.tile([P, 1], F32)
            act_scratch = psum.tile([P, D], F32)  # ACT Square dump
            dve_scratch = singles.tile([P, D], F32)  # DVE stt dump

            def process_group(g, t_base, src_ap, gg, n_dve):
                xg = xpool.tile([P, gg, D], F32, tag="xg", name=f"xg{g}")
                nc.sync.dma_start(out=xg[:], in_=src_ap)

                xbf = xpool.tile([P, gg, D], BF16, tag="xbf", name=f"xbf{g}")
                nc.vector.tensor_copy(xbf[:], xg[:])

                # per-row sum of squares, split ACT / DVE to balance load
                ssg = small.tile([P, gg], F32, tag="ssg", name=f"ssg{g}")
                for j in range(gg):
                    if j >= gg - n_dve:
                        nc.vector.scalar_tensor_tensor(
                            out=dve_scratch[:],
                            in0=xg[:, j],
                            scalar=1.0,
                            in1=xg[:, j],
                            op0=OP.mult,
                            op1=OP.mult,
                            accum_out=ssg[:, j : j + 1],
                        )
                    else:
                        nc.scalar.activation(
                            out=act_scratch[:],
                            in_=xg[:, j],
                            func=AF.Square,
                            accum_out=ssg[:, j : j + 1],
                        )

                # rnorm = 1/sqrt(max(ss, eps)), Newton-refined; ncol = ss*rnorm
                def st(nm):
                    return small.tile([P, gg], F32, tag=nm, name=f"{nm}{g}")

                ssc = st("ssc")
                nc.vector.tensor_scalar_max(ssc[:], ssg[:], 1e-12)
                sqg = st("sqg")
                nc.scalar.activation(out=sqg[:], in_=ssc[:], func=AF.Sqrt)
                r0 = st("r0")
                nc.vector.reciprocal(r0[:], sqg[:])
                t0 = st("t0")
                nc.vector.tensor_mul(t0[:], r0[:], r0[:])
                t1 = st("t1")
                nc.vector.tensor_mul(t1[:], t0[:], ssc[:])
                t2 = st("t2")
                nc.vector.tensor_scalar(t2[:], t1[:], -0.5, 1.5, OP.mult, OP.add)
                rn = st("rn")
                nc.vector.tensor_mul(rn[:], r0[:], t2[:])
                ncbf = small.tile([P, gg], BF16, tag="ncbf", name=f"ncbf{g}")
                nc.vector.tensor_mul(ncbf[:], ssc[:], rn[:])

                for j in range(gg):
                    t = t_base + j
                    oh = ohpool.tile([P, CLOC], BF16, tag="oh", name=f"oh{t}")
                    nc.vector.tensor_scalar(
                        oh[:],
                        iota_sb[:],
                        lab_sb[:, t : t + 1],
                        rn[:, j : j + 1],
                        OP.is_equal,
                        OP.mult,
                    )
                    nc.tensor.matmul(
                        psum_sums[:],
                        lhsT=oh[:],
                        rhs=xbf[:, j],
                        start=(t == 0),
                        stop=(t == NT - 1),
                    )
                    nc.tensor.matmul(
                        psum_cnt[:],
                        lhsT=oh[:],
                        rhs=ncbf[:, j : j + 1],
                        start=(t == 0),
                        stop=(t == NT - 1),
                    )

            for g in range(NG):
                process_group(
                    g, g * G, x_ext[g], G, n_dve=(1 if (g % 2 == 0) else 2)
                )
            process_group(NG, NG * G, xt_ext[:, :, :], G_TAIL, n_dve=1)

            # ---- epilogue: per-class loss from sums/counts ----
            sums_sb = singles.tile([P, D], F32)
            nc.vector.tensor_copy(sums_sb[:], psum_sums[:])
            cnt = singles.tile([P, 1], F32)
            nc.vector.tensor_copy(cnt[:], psum_cnt[:])

            colsum = singles.tile([P, 1], F32)
            nc.vector.tensor_reduce(
                colsum[:], sums_sb[:], mybir.AxisListType.X, OP.add
            )
            junk = singles.tile([P, D], F32)
            nc.vector.tensor_mul(junk[:], sums_sb[:], sums_sb[:])
            sumsq = singles.tile([P, 1], F32)
            nc.vector.tensor_reduce(
                sumsq[:], junk[:], mybir.AxisListType.X, OP.add
            )

            _ep_n = [0]

            def newt():
                _ep_n[0] += 1
                return singles.tile(
                    [P, 1], F32, name=f"ep{_ep_n[0]}", tag=f"ep{_ep_n[0]}"
                )

            s0 = newt()
            nc.vector.tensor_scalar_max(s0[:], sumsq[:], 1e-20)
            sq2 = newt()
            nc.scalar.activation(out=sq2[:], in_=s0[:], func=AF.Sqrt)
            r0e = newt()
            nc.vector.reciprocal(r0e[:], sq2[:])
            a0 = newt()
            nc.vector.tensor_mul(a0[:], r0e[:], r0e[:])
            a1 = newt()
            nc.vector.tensor_mul(a1[:], a0[:], s0[:])
            a2 = newt()
            nc.vector.tensor_scalar(a2[:], a1[:], -0.5, 1.5, OP.mult, OP.add)
            ri = newt()
            nc.vector.tensor_mul(ri[:], r0e[:], a2[:])
            normS = newt()
            nc.vector.tensor_mul(normS[:], s0[:], ri[:])
            mask = newt()
            nc.vector.tensor_scalar(mask[:], sumsq[:], 1e-12, None, OP.is_gt)
            sm = newt()
            nc.vector.tensor_mul(sm[:], colsum[:], ri[:])
            S = newt()
            nc.vector.tensor_mul(S[:], sm[:], mask[:])
            l1 = newt()
            nc.vector.tensor_scalar_mul(l1[:], cnt[:], K_CONST)
            l2 = newt()
            nc.vector.tensor_scalar_mul(l2[:], colsum[:], INV_D5)
            lseg = newt()
            nc.vector.tensor_add(lseg[:], l1[:], l2[:])
            aa = newt()
            nc.vector.tensor_mul(aa[:], S[:], lseg[:])
            bb = newt()
            nc.vector.tensor_mul(bb[:], normS[:], mask[:])
            nbb = newt()
            nc.vector.tensor_scalar_mul(nbb[:], bb[:], -1.0)
            num = newt()
            nc.vector.tensor_add(num[:], aa[:], nbb[:])
            cc = newt()
            nc.vector.tensor_scalar_max(cc[:], cnt[:], 1.0)
            ic = newt()
            nc.vector.reciprocal(ic[:], cc[:])
            loss = newt()
            nc.vector.tensor_mul(loss[:], num[:], ic[:])

            nc.sync.dma_start(out=out_ext[:, :], in_=loss[:])

    nc.compile()
    return nc


def assign_classes(labels):
    """Greedy balanced partition: 128 classes per core, near-equal row totals.
    Returns (owner_of_cls [C], pos_of_cls [C], cls_at [NCORES, CLOC])."""
    counts = np.bincount(labels, minlength=C)
    order = np.argsort(-counts, kind="stable")
    bin_rows = np.zeros(NCORES, dtype=np.int64)
    bin_n = np.zeros(NCORES, dtype=np.int64)
    owner_of_cls = np.empty(C, dtype=np.int64)
    pos_of_cls = np.empty(C, dtype=np.int64)
    cls_at = np.empty((NCORES, CLOC), dtype=np.int64)
    for cidx in order:
        open_bins = np.flatnonzero(bin_n < CLOC)
        k = open_bins[np.argmin(bin_rows[open_bins])]
        owner_of_cls[cidx] = k
        pos_of_cls[cidx] = bin_n[k]
        cls_at[k, bin_n[k]] = cidx
        bin_n[k] += 1
        bin_rows[k] += counts[cidx]
    return owner_of_cls, pos_of_cls, cls_at, bin_rows


def make_in_maps(logits, labels):
    """Host-side sharding: route each row to the core owning its (balanced)
    class bin; lay X out so each partition's per-group data is contiguous."""
    logits = np.ascontiguousarray(np.asarray(logits, dtype=np.float32))
    labels = np.asarray(labels).astype(np.int64)
    owner_of_cls, pos_of_cls, cls_at, bin_rows = assign_classes(labels)
    assert bin_rows.max() <= CAP, f"max shard {bin_rows.max()} > capacity {CAP}"
    owner = owner_of_cls[labels]
    local = pos_of_cls[labels]
    in_maps = []
    iota_tile = np.ascontiguousarray(
        np.broadcast_to(
            np.arange(CLOC, dtype=np.float32).astype(ml_dtypes.bfloat16),
            (P, CLOC),
        )
    )
    for k in range(NCORES):
        idx = np.flatnonzero(owner == k)
        nk = idx.size
        xs = np.zeros((CAP, D), dtype=np.float32)
        xs[:nk] = logits[idx]
        # full groups: row (g*G + j)*P + p -> x4[g, p, j, :]
        x4 = np.ascontiguousarray(
            xs[: NG * G * P].reshape(NG, G, P, D).transpose(0, 2, 1, 3)
        )
        xt = np.ascontiguousarray(
            xs[NG * G * P :].reshape(G_TAIL, P, D).transpose(1, 0, 2)
        )
        ll = np.full((CAP,), -1.0, dtype=np.float32)
        ll[:nk] = local[idx].astype(np.float32)
        lab2d = np.ascontiguousarray(ll.reshape(NT, P).T)  # [p, t] = ll[t*128+p]
        in_maps.append(
            {"x": x4, "xt": xt, "lab": lab2d, "iota": iota_tile}
        )
    return in_maps, cls_at


_NC_CACHE = {}


def get_nc():
    if "nc" not in _NC_CACHE:
        _NC_CACHE["nc"] = build_nc()
    return _NC_CACHE["nc"]


def run(logits, labels, num_classes, trace=False, **spmd_kwargs):
    assert int(num_classes) == C
    nc = get_nc()
    in_maps, cls_at = make_in_maps(logits, labels)
    res = run_bass_kernel_spmd(
        nc, in_maps, core_ids=list(range(NCORES)), trace=trace, **spmd_kwargs
    )
    out = np.empty((C,), dtype=np.float32)
    for k in range(NCORES):
        out[cls_at[k]] = res.results[k]["out"].ravel()
    return out, res


def kernel(logits, labels, num_classes):
    out, _ = run(logits, labels, num_classes)
    return out
